# revision 18
# baseline (speedup 1.0000x reference)
"""Trainium2 Bass kernel: bilinear interpolation from BEV feature maps.

reference semantics (interpolate_from_bev_features, correction=False):
  keypoints (B, N, 3) f32; bev_features (B, C, H, W) f32; bev_stride scalar
  out (B, N, C) f32: bilinear sample at x = kp_x/(0.05*stride),
  y = (kp_y+40)/(0.05*stride); corner indices clamped to [0, 187]; weights
  from clamped corner coords (out-of-range y yields exact 0).

Sharding: 8 cores = batch (4) x channel-half (2). Each core keeps its
(b, ch) 128-channel slab in SBUF split into even/odd-row halves (the
GPSIMD ap_gather window is capped at 2^15 elements), gathers the four
bilinear corners per keypoint with ap_gather (d=1), blends on DVE against
PE-broadcast per-keypoint weights held in PSUM, transposes 128x128 blocks
on PE, and writes its (N, 128) output shard.

Row-pair parity trick: the corner rows are (r, r+1) with r = min(floor(y),
186) - one even, one odd - so the "even-row" gather always reads the even
slab and the "odd-row" gather the odd slab; which of wy0/wy1 applies is
selected arithmetically by the parity of r. Clamped keypoints
(floor(y) >= 187) contribute exactly 0, enforced with a 0/1 mask.

Shapes hardcoded per problem spec: B=4 N=4096 C=256 H=W=188, stride 8.
"""
import os
import sys

for _p in ('/opt/trn_rl_repo', '/root/.axon_site/_ro/trn_rl_repo'):
    if os.path.isdir(_p) and _p not in sys.path:
        sys.path.append(_p)

import numpy as np

B, N, C, H, W = 4, 4096, 256, 188, 188
W_PACK = 178           # x <= 176 -> x1 <= 177; cols 178..187 never read
HHALF = H // 2         # 94 rows per parity slab
FLAT_H = HHALF * W_PACK        # 16732 elements per half-slab
NDMA_H = 8                     # DMA chunks per half-slab load
FLAT_H_PAD = 16736             # 8 x 2092
CHUNK = 256                    # keypoints per pipeline chunk
NCHUNK = N // CHUNK            # 16
N_CORES = 8

_compiled = {}


def _build(scale: float, ybias: float, debug_taps: bool = False):
    import concourse.bacc as bacc
    import concourse.mybir as mybir
    import concourse.tile as tile
    import contextlib
    from concourse.masks import make_identity

    dt = mybir.dt
    nc = bacc.Bacc("TRN2", target_bir_lowering=False, debug=False,
                   num_devices=N_CORES)

    slabe_d = nc.dram_tensor("slabe", [128, FLAT_H_PAD], dt.float32, kind="ExternalInput")
    slabo_d = nc.dram_tensor("slabo", [128, FLAT_H_PAD], dt.float32, kind="ExternalInput")
    kp_d = nc.dram_tensor("kp", [N, 3], dt.float32, kind="ExternalInput")
    out_d = nc.dram_tensor("out", [N, 128], dt.float32, kind="ExternalOutput")
    taps = {}
    if debug_taps:
        for nm, shp in [("t_X0", [128, 32]), ("t_Y0R", [128, 32]), ("t_M", [128, 32]),
                        ("t_P", [128, 32]), ("t_WEL", [128, 32]), ("t_WOR", [128, 32]),
                        ("t_IEL", [128, 256]), ("t_R4", [1, 1024]),
                        ("t_GEL", [128, 256]), ("t_OUTC", [128, 256])]:
            taps[nm] = nc.dram_tensor(nm, shp, dt.float32, kind="ExternalOutput")

    kp_nat = kp_d.ap().rearrange("(p s) c -> p (s c)", p=128)       # [128, 96]
    kp_wrp = kp_d.ap().rearrange("(s r) c -> r s c", r=16)          # [16, 256, 3]
    out_r = out_d.ap().rearrange("(c j p) d -> c p j d", j=2, p=128)

    AF = mybir.ActivationFunctionType
    OP = mybir.AluOpType

    with tile.TileContext(nc) as tc, contextlib.ExitStack() as ctx:
        slab_pool = ctx.enter_context(tc.tile_pool(name="slab", bufs=1))
        meta = ctx.enter_context(tc.tile_pool(name="meta", bufs=1))
        rows = ctx.enter_context(tc.tile_pool(name="rows", bufs=2))
        gat = ctx.enter_context(tc.tile_pool(name="gat", bufs=3))
        blend = ctx.enter_context(tc.tile_pool(name="blend", bufs=2))
        stage = ctx.enter_context(tc.tile_pool(name="stage", bufs=2))
        psum = ctx.enter_context(tc.tile_pool(name="psum", bufs=2, space="PSUM"))

        # ---- slab loads (split across DMA queues) ----
        SLAB_E = slab_pool.tile([128, FLAT_H_PAD, 1], dt.float32)
        SLAB_O = slab_pool.tile([128, FLAT_H_PAD, 1], dt.float32)
        step = FLAT_H_PAD // NDMA_H
        for tile_, dram_ in ((SLAB_E, slabe_d), (SLAB_O, slabo_d)):
            flat_v = tile_[:].rearrange("p a b -> p (a b)")
            for k in range(NDMA_H):
                nc.sync.dma_start(flat_v[:, k * step:(k + 1) * step],
                                  dram_.ap()[:, k * step:(k + 1) * step])

        def floor_of(v_ap, pool, nfree, tag):
            """floor(v) for v >= 0, exact under either trunc or round-to-
            nearest f32->i32 cast: c = float(int(v)); c -= (c > v)."""
            CI = pool.tile([128, nfree], dt.int32, tag=tag + "i")
            nc.vector.tensor_copy(out=CI[:], in_=v_ap)
            CF = pool.tile([128, nfree], dt.float32, tag=tag + "f")
            nc.vector.tensor_copy(out=CF[:], in_=CI[:])
            GT = pool.tile([128, nfree], dt.float32, tag=tag + "g")
            nc.vector.tensor_tensor(GT[:], CF[:], v_ap, op=OP.is_gt)
            OUT = pool.tile([128, nfree], dt.float32, tag=tag + "o")
            nc.vector.tensor_tensor(OUT[:], CF[:], GT[:], op=OP.subtract)
            return OUT

        def coords(x_ap, y_ap, pool, nfree, pfx):
            """-> (XS, YS, X0, Y0R, P) f32 [128, nfree]: scaled coords,
            x floor, row-pair base min(floor(y),186), base parity."""
            XS = pool.tile([128, nfree], dt.float32, tag=pfx + "XS")
            nc.scalar.activation(XS[:], x_ap, AF.Copy, bias=0.0, scale=scale)
            YS = pool.tile([128, nfree], dt.float32, tag=pfx + "YS")
            nc.scalar.activation(YS[:], y_ap, AF.Copy, bias=ybias, scale=scale)
            X0 = floor_of(XS[:], pool, nfree, pfx + "fx")
            T = floor_of(YS[:], pool, nfree, pfx + "fy")
            Y0R = pool.tile([128, nfree], dt.float32, tag=pfx + "Y0R")
            nc.vector.tensor_scalar(Y0R[:], T[:], float(H - 2), None, OP.min)
            YH = pool.tile([128, nfree], dt.float32, tag=pfx + "YH")
            nc.vector.tensor_scalar(YH[:], Y0R[:], 0.5, None, OP.mult)
            YHF = floor_of(YH[:], pool, nfree, pfx + "fh")
            P = pool.tile([128, nfree], dt.float32, tag=pfx + "P")
            nc.vector.tensor_scalar(P[:], YHF[:], -2.0, None, OP.mult)
            nc.vector.tensor_tensor(P[:], Y0R[:], P[:], op=OP.add)  # y0r - 2*floor(y0r/2)
            return XS, YS, X0, Y0R, P, T

        KP = meta.tile([128, 96], dt.float32)
        nc.sync.dma_start(KP[:], kp_nat)
        kp3 = KP[:].rearrange("p (s c) -> p s c", c=3)
        XS, YS, X0, Y0R, PAR, TT = coords(kp3[:, :, 0], kp3[:, :, 1], meta, 32, "n")

        FX = meta.tile([128, 32], dt.float32)
        nc.vector.tensor_tensor(FX[:], XS[:], X0[:], op=OP.subtract)       # x - x0f
        WXL = meta.tile([128, 32], dt.float32)
        nc.vector.tensor_scalar(WXL[:], FX[:], 1.0, -1.0, OP.subtract, OP.mult)  # 1-fx
        M = meta.tile([128, 32], dt.float32)
        nc.vector.tensor_scalar(M[:], TT[:], float(H - 2), None, OP.is_le)  # floor(y)<=186
        WY0 = meta.tile([128, 32], dt.float32)   # (y0r+1) - ys, masked
        nc.vector.tensor_scalar(WY0[:], Y0R[:], 1.0, None, OP.add)
        nc.vector.tensor_tensor(WY0[:], WY0[:], YS[:], op=OP.subtract)
        nc.vector.tensor_tensor(WY0[:], WY0[:], M[:], op=OP.mult)
        WY1 = meta.tile([128, 32], dt.float32)   # ys - y0r, masked
        nc.vector.tensor_tensor(WY1[:], YS[:], Y0R[:], op=OP.subtract)
        nc.vector.tensor_tensor(WY1[:], WY1[:], M[:], op=OP.mult)
        # parity select: wE = wy0 + p*(wy1-wy0); wO = wy1 - p*(wy1-wy0)
        DWY = meta.tile([128, 32], dt.float32)
        nc.vector.tensor_tensor(DWY[:], WY1[:], WY0[:], op=OP.subtract)
        nc.vector.tensor_tensor(DWY[:], DWY[:], PAR[:], op=OP.mult)
        WE = meta.tile([128, 32], dt.float32)
        nc.vector.tensor_tensor(WE[:], WY0[:], DWY[:], op=OP.add)
        WO = meta.tile([128, 32], dt.float32)
        nc.vector.tensor_tensor(WO[:], WY1[:], DWY[:], op=OP.subtract)
        W4 = []
        for nm, wy, wx in (("WEL", WE, WXL), ("WER", WE, FX),
                           ("WOL", WO, WXL), ("WOR", WO, FX)):
            t = meta.tile([128, 32], dt.float32, tag=nm)
            nc.vector.tensor_tensor(t[:], wy[:], wx[:], op=OP.mult)
            W4.append(t)

        # ---- wrapped-16 pipeline: gather indices ----
        KPW = meta.tile([128, 768], dt.float32)
        KPW3v = KPW[:].rearrange("p (s c) -> p s c", c=3)
        for g in range(8):
            nc.sync.dma_start(KPW3v[g * 16:(g + 1) * 16], kp_wrp)
        kpw3 = KPW[:].rearrange("p (s c) -> p s c", c=3)
        _, _, X0w, Y0Rw, PARw, _ = coords(kpw3[:, :, 0], kpw3[:, :, 1], meta, 256, "w")
        # he = (y0r+p)/2 ; ho = (y0r-p)/2
        HE = meta.tile([128, 256], dt.float32)
        nc.vector.tensor_tensor(HE[:], Y0Rw[:], PARw[:], op=OP.add)
        nc.vector.tensor_scalar(HE[:], HE[:], 0.5, None, OP.mult)
        HO = meta.tile([128, 256], dt.float32)
        nc.vector.tensor_tensor(HO[:], Y0Rw[:], PARw[:], op=OP.subtract)
        nc.vector.tensor_scalar(HO[:], HO[:], 0.5, None, OP.mult)
        IDX = []
        for nm, hh in (("E", HE), ("O", HO)):
            base = meta.tile([128, 256], dt.float32, tag="base" + nm)
            nc.vector.tensor_scalar(base[:], hh[:], float(W_PACK), None, OP.mult)
            nc.vector.tensor_tensor(base[:], base[:], X0w[:], op=OP.add)
            il = meta.tile([128, 256], dt.int16, tag="I" + nm + "L")
            nc.vector.tensor_copy(out=il[:], in_=base[:])
            baser = meta.tile([128, 256], dt.float32, tag="baser" + nm)
            nc.vector.tensor_scalar(baser[:], base[:], 1.0, None, OP.add)
            ir = meta.tile([128, 256], dt.int16, tag="I" + nm + "R")
            nc.vector.tensor_copy(out=ir[:], in_=baser[:])
            IDX.extend([il, ir])
        IEL, IER, IOL, IOR = IDX

        ONES = meta.tile([1, 128], dt.float32)
        nc.vector.memset(ONES[:], 1.0)
        IDENT = meta.tile([128, 128], dt.float32)
        make_identity(nc, IDENT[:])

        if debug_taps:
            nc.sync.dma_start(taps["t_X0"].ap(), X0[:])
            nc.sync.dma_start(taps["t_Y0R"].ap(), Y0R[:])
            nc.sync.dma_start(taps["t_M"].ap(), M[:])
            nc.sync.dma_start(taps["t_P"].ap(), PAR[:])
            nc.sync.dma_start(taps["t_WEL"].ap(), W4[0][:])
            nc.sync.dma_start(taps["t_WOR"].ap(), W4[3][:])
            IELf = meta.tile([128, 256], dt.float32)
            nc.vector.tensor_copy(out=IELf[:], in_=IEL[:])
            nc.sync.dma_start(taps["t_IEL"].ap(), IELf[:])

        # ---- per-chunk gather + blend + transpose + store ----
        for c in range(NCHUNK):
            p0 = c * (CHUNK // 32)          # 8 natural partitions per chunk
            R4 = rows.tile([1, 4 * CHUNK], dt.float32, tag="R4")
            for q in range(4):
                nc.sync.dma_start(R4[:, q * CHUNK:(q + 1) * CHUNK],
                                  W4[q][p0:p0 + 8, :])
            Wps = psum.tile([128, 4 * CHUNK], dt.float32, space="PSUM", tag="Wps")
            nc.tensor.matmul(Wps[:, 0:512], lhsT=ONES[:], rhs=R4[:, 0:512],
                             start=True, stop=True)
            nc.tensor.matmul(Wps[:, 512:1024], lhsT=ONES[:], rhs=R4[:, 512:1024],
                             start=True, stop=True)

            G4 = []
            for nm, slab_, idx_ in (("GEL", SLAB_E, IEL), ("GER", SLAB_E, IER),
                                    ("GOL", SLAB_O, IOL), ("GOR", SLAB_O, IOR)):
                g = gat.tile([128, CHUNK, 1], dt.float32, tag=nm)
                nc.gpsimd.ap_gather(g[:], slab_[:], idx_[:, c * 16:(c + 1) * 16],
                                    channels=128, num_elems=FLAT_H_PAD, d=1,
                                    num_idxs=CHUNK)
                G4.append(g)

            PRD = []
            for q in range(4):
                t = blend.tile([128, CHUNK], dt.float32, tag=f"PRD{q}")
                nc.vector.tensor_tensor(t[:], G4[q][:, :, 0],
                                        Wps[:, q * CHUNK:(q + 1) * CHUNK], op=OP.mult)
                PRD.append(t)
            S1 = blend.tile([128, CHUNK], dt.float32, tag="S1")
            nc.vector.tensor_tensor(S1[:], PRD[0][:], PRD[1][:], op=OP.add)
            S2 = blend.tile([128, CHUNK], dt.float32, tag="S2")
            nc.vector.tensor_tensor(S2[:], PRD[2][:], PRD[3][:], op=OP.add)
            OUTC = blend.tile([128, CHUNK], dt.float32, tag="OUTC")
            nc.vector.tensor_tensor(OUTC[:], S1[:], S2[:], op=OP.add)

            if debug_taps and c == 0:
                nc.sync.dma_start(taps["t_R4"].ap(), R4[:])
                nc.sync.dma_start(taps["t_GEL"].ap(),
                                  G4[0][:].rearrange("p a b -> p (a b)"))
                nc.sync.dma_start(taps["t_OUTC"].ap(), OUTC[:])

            OS = stage.tile([128, 2, 128], dt.float32, tag="OS")
            for j in range(2):
                TP = psum.tile([128, 128], dt.float32, space="PSUM", tag="TP")
                nc.tensor.transpose(TP[:], OUTC[:, j * 128:(j + 1) * 128], IDENT[:])
                nc.scalar.copy(OS[:, j, :], TP[:])
            nc.sync.dma_start(out_r[c], OS[:])

    nc.compile()
    return nc


def _get(scale: float, ybias: float):
    key = (round(scale, 9), round(ybias, 9))
    if key not in _compiled:
        _compiled[key] = _build(scale, ybias)
    return _compiled[key]


def _prepare_in_maps(keypoints: np.ndarray, bev_features: np.ndarray):
    kp = np.ascontiguousarray(keypoints, dtype=np.float32)
    bev = np.asarray(bev_features, dtype=np.float32)
    in_maps = []
    for core in range(N_CORES):
        b, ch = core // 2, core % 2
        sl = slice(ch * 128, (ch + 1) * 128)
        se = np.zeros((128, FLAT_H_PAD), dtype=np.float32)
        se[:, :FLAT_H] = bev[b, sl, 0::2, :W_PACK].reshape(128, FLAT_H)
        so = np.zeros((128, FLAT_H_PAD), dtype=np.float32)
        so[:, :FLAT_H] = bev[b, sl, 1::2, :W_PACK].reshape(128, FLAT_H)
        in_maps.append({"slabe": se, "slabo": so, "kp": kp[b]})
    return in_maps


def _assemble(results) -> np.ndarray:
    out = np.empty((B, N, C), dtype=np.float32)
    for core in range(N_CORES):
        b, ch = core // 2, core % 2
        out[b, :, ch * 128:(ch + 1) * 128] = np.asarray(results[core]["out"])
    return out


def _scale_bias(bev_stride):
    stride = float(np.asarray(bev_stride))
    scale = 1.0 / (0.05 * stride)
    return scale, 40.0 * scale


def kernel(keypoints: np.ndarray, bev_features: np.ndarray, bev_stride) -> np.ndarray:
    from concourse.bass_utils import run_bass_kernel_spmd

    scale, ybias = _scale_bias(bev_stride)
    nc = _get(scale, ybias)
    in_maps = _prepare_in_maps(keypoints, bev_features)
    res = run_bass_kernel_spmd(nc, in_maps, list(range(N_CORES))).results
    return _assemble(res)


# revision 26
# speedup vs baseline: 1.8994x; 1.8994x over previous
"""Trainium2 Bass kernel: bilinear interpolation from BEV feature maps.

reference semantics (interpolate_from_bev_features, correction=False):
  keypoints (B, N, 3) f32; bev_features (B, C, H, W) f32; bev_stride scalar
  out (B, N, C) f32: bilinear sample at x = kp_x/(0.05*stride),
  y = (kp_y+40)/(0.05*stride); corner indices clamped to [0, 187]; weights
  from clamped corner coords (out-of-range y cancels to exactly 0).

Sharding: 8 cores = batch (4) x channel-half (2).

Per-core plan (SBUF gather ucode measured ~20 ns/element on this part, so
the gather runs on the DMA engines instead):
  Phase A: stream-transpose the (128ch, H*W_PACK) slab into a DRAM scratch
    TBEV[px, 128ch]: DMA load [128, 2048] -> DVE 32x32 stream-transpose ->
    4 DMA stores with block-permuted 3D access patterns (128B runs).
  Phase B: dma_gather (MoE-style SWDGE gather) fetches, per keypoint corner
    row, a 384-element run (3 pixels x 128ch starting at the even pixel
    below x0) out of an overlapped [V, 384]/stride-256 view of TBEV.
    int16 gather indices address 256-element pair rows (max 16731).
    The x-parity selects which 2 of the 3 pixels matter - folded into
    per-keypoint 3-slot weights, applied on DVE via stride-0 broadcast.
    Output lands keypoint-major: straight DMA out.

Shapes hardcoded per problem spec: B=4 N=4096 C=256 H=W=188 (x<=176 so
only W_PACK=178 columns are ever addressed).
"""
import os
import sys

for _p in ('/opt/trn_rl_repo', '/root/.axon_site/_ro/trn_rl_repo'):
    if os.path.isdir(_p) and _p not in sys.path:
        sys.path.append(_p)

import numpy as np

B, N, C, H, W = 4, 4096, 256, 188, 188
W_PACK = 178                  # x <= 176 -> x1 <= 177; cols 178..187 never read
FLAT = H * W_PACK             # 33464 pixels
NLOAD = 17                    # phase-A loads of [128, 2048]
FLAT_PAD = NLOAD * 2048       # 34816
VPAIR = FLAT_PAD * 128 // 256 - 1   # overlapped 384-elem rows, stride 256
BCH = 1024                    # gather indices per dma_gather call
NBCH = N // BCH               # 4 phase-B chunks
GPC = BCH // 128              # keypoint blocks of 128 per chunk (8)
N_CORES = 8

_compiled = {}


def _build(scale: float, ybias: float, debug_taps: bool = False):
    import concourse.bacc as bacc
    import concourse.mybir as mybir
    import concourse.tile as tile
    import contextlib
    from concourse.bass import AP

    dt = mybir.dt
    nc = bacc.Bacc("TRN2", target_bir_lowering=False, debug=False,
                   num_devices=N_CORES)

    slab_d = nc.dram_tensor("slab", [128, FLAT_PAD], dt.float32, kind="ExternalInput")
    kp_d = nc.dram_tensor("kp", [N, 3], dt.float32, kind="ExternalInput")
    out_d = nc.dram_tensor("out", [N, 128], dt.float32, kind="ExternalOutput")

    taps = {}
    if debug_taps:
        for nm, shp in [("t_X0", [128, 32]), ("t_QX", [128, 32]),
                        ("t_W3AC", [128, 96]), ("t_I0w", [128, 256]),
                        ("t_G0", [128, 384]), ("t_TB", [128, 128])]:
            taps[nm] = nc.dram_tensor(nm, shp, dt.float32, kind="ExternalOutput")

    # keypoint n = s*128 + p  (block-major) for weights;
    # n = s*16 + r (wrapped-16) for gather indices
    kp_blk = kp_d.ap().rearrange("(s p) c -> p s c", p=128)        # [128, 32, 3]
    kp_wrp = kp_d.ap().rearrange("(s r) c -> r s c", r=16)         # [16, 256, 3]
    out_r = out_d.ap().rearrange("(cb g p) c -> cb p g c", g=GPC, p=128)

    AF = mybir.ActivationFunctionType
    OP = mybir.AluOpType

    with tile.TileContext(nc) as tc, contextlib.ExitStack() as ctx:
        lda = ctx.enter_context(tc.tile_pool(name="lda", bufs=3))
        tta = ctx.enter_context(tc.tile_pool(name="tta", bufs=3))
        meta = ctx.enter_context(tc.tile_pool(name="meta", bufs=1))
        gat = ctx.enter_context(tc.tile_pool(name="gat", bufs=2))
        blend = ctx.enter_context(tc.tile_pool(name="blend", bufs=2))
        dram = ctx.enter_context(tc.tile_pool(name="dram", bufs=1, space="DRAM"))
        TB = dram.tile([FLAT_PAD, 128], dt.float32)

        # ---- phase A: slab[c, px] -> TBEV[px, c] ----
        # StreamTranspose: TTB[32a+r, 32m+s] = BLK[32a+s, 32m+r]; one store
        # per 32-channel group a keeps both DMA access patterns at 3 dims.
        for kb in range(NLOAD):
            BLK = lda.tile([128, 2048], dt.float32, tag="BLK")
            nc.sync.dma_start(BLK[:], slab_d.ap()[:, kb * 2048:(kb + 1) * 2048])
            TTB = tta.tile([128, 2048], dt.float32, tag="TTB")
            nc.vector.transpose(TTB[:], BLK[:])
            for a in range(4):
                dst = TB[kb * 2048:(kb + 1) * 2048, a * 32:(a + 1) * 32] \
                    .rearrange("(m r) s -> r m s", r=32)
                src = TTB[a * 32:(a + 1) * 32, :].rearrange("p (m s) -> p m s", s=32)
                eng = nc.sync if a % 2 == 0 else nc.scalar
                eng.dma_start(dst, src)

        # ---- keypoint math ----
        def floor_of(v_ap, pool, nfree, tag):
            """floor(v) for v >= 0, exact under trunc or round f32<->i32."""
            CI = pool.tile([128, nfree], dt.int32, tag=tag + "i")
            nc.vector.tensor_copy(out=CI[:], in_=v_ap)
            CF = pool.tile([128, nfree], dt.float32, tag=tag + "f")
            nc.vector.tensor_copy(out=CF[:], in_=CI[:])
            GT = pool.tile([128, nfree], dt.float32, tag=tag + "g")
            nc.vector.tensor_tensor(GT[:], CF[:], v_ap, op=OP.is_gt)
            OUT = pool.tile([128, nfree], dt.float32, tag=tag + "o")
            nc.vector.tensor_tensor(OUT[:], CF[:], GT[:], op=OP.subtract)
            return OUT

        def coords(x_ap, y_ap, nfree, pfx):
            """-> (XS, YS, X0, Y0, Y1) f32 [128, nfree], reference clamps."""
            XS = meta.tile([128, nfree], dt.float32, tag=pfx + "XS")
            nc.scalar.activation(XS[:], x_ap, AF.Copy, bias=0.0, scale=scale)
            YS = meta.tile([128, nfree], dt.float32, tag=pfx + "YS")
            nc.scalar.activation(YS[:], y_ap, AF.Copy, bias=ybias, scale=scale)
            X0 = floor_of(XS[:], meta, nfree, pfx + "fx")
            T = floor_of(YS[:], meta, nfree, pfx + "fy")
            Y0 = meta.tile([128, nfree], dt.float32, tag=pfx + "Y0")
            nc.vector.tensor_scalar(Y0[:], T[:], float(H - 1), None, OP.min)
            Y1 = meta.tile([128, nfree], dt.float32, tag=pfx + "Y1")
            nc.vector.tensor_scalar(Y1[:], T[:], 1.0, float(H - 1), OP.add, OP.min)
            return XS, YS, X0, Y0, Y1

        # block-major pipeline: weights
        KP = meta.tile([128, 96], dt.float32)
        kp3 = KP[:].rearrange("p (s c) -> p s c", c=3)
        nc.sync.dma_start(kp3, kp_blk)
        XS, YS, X0, Y0, Y1 = coords(kp3[:, :, 0], kp3[:, :, 1], 32, "n")

        FX = meta.tile([128, 32], dt.float32)
        nc.vector.tensor_tensor(FX[:], XS[:], X0[:], op=OP.subtract)
        WXL = meta.tile([128, 32], dt.float32)
        nc.vector.tensor_scalar(WXL[:], FX[:], 1.0, -1.0, OP.subtract, OP.mult)
        WY0 = meta.tile([128, 32], dt.float32)
        nc.vector.tensor_tensor(WY0[:], Y1[:], YS[:], op=OP.subtract)
        WY1 = meta.tile([128, 32], dt.float32)
        nc.vector.tensor_tensor(WY1[:], YS[:], Y0[:], op=OP.subtract)
        # x parity qx = x0 mod 2 (row base y*178 is even)
        XH = meta.tile([128, 32], dt.float32)
        nc.vector.tensor_scalar(XH[:], X0[:], 0.5, None, OP.mult)
        XHF = floor_of(XH[:], meta, 32, "nqh")
        QX = meta.tile([128, 32], dt.float32)
        nc.vector.tensor_scalar(QX[:], XHF[:], -2.0, None, OP.mult)
        nc.vector.tensor_tensor(QX[:], X0[:], QX[:], op=OP.add)
        QM = meta.tile([128, 32], dt.float32)
        nc.vector.tensor_scalar(QM[:], QX[:], 1.0, -1.0, OP.subtract, OP.mult)
        # 3-slot x weights: u0 = wxl*(1-qx); u1 = wxl*qx + fx*(1-qx); u2 = fx*qx
        U0 = meta.tile([128, 32], dt.float32)
        nc.vector.tensor_tensor(U0[:], WXL[:], QM[:], op=OP.mult)
        U1 = meta.tile([128, 32], dt.float32)
        T1 = meta.tile([128, 32], dt.float32)
        nc.vector.tensor_tensor(T1[:], WXL[:], QX[:], op=OP.mult)
        nc.vector.tensor_tensor(U1[:], FX[:], QM[:], op=OP.mult)
        nc.vector.tensor_tensor(U1[:], U1[:], T1[:], op=OP.add)
        U2 = meta.tile([128, 32], dt.float32)
        nc.vector.tensor_tensor(U2[:], FX[:], QX[:], op=OP.mult)
        W3AC = meta.tile([128, 32, 3], dt.float32)
        W3BD = meta.tile([128, 32, 3], dt.float32)
        for k, u in enumerate((U0, U1, U2)):
            nc.vector.tensor_tensor(W3AC[:, :, k], u[:], WY0[:], op=OP.mult)
            nc.vector.tensor_tensor(W3BD[:, :, k], u[:], WY1[:], op=OP.mult)

        # wrapped-16 pipeline: gather pair-row indices (int16)
        KPW = meta.tile([128, 768], dt.float32)
        kpw3 = KPW[:].rearrange("p (s c) -> p s c", c=3)
        for g in range(8):
            nc.sync.dma_start(kpw3[g * 16:(g + 1) * 16], kp_wrp)
        _, _, X0w, Y0w, Y1w = coords(kpw3[:, :, 0], kpw3[:, :, 1], 256, "w")
        IDXW = []
        for nm, yy in (("I0", Y0w), ("I1", Y1w)):
            base = meta.tile([128, 256], dt.float32, tag=nm + "b")
            nc.vector.tensor_scalar(base[:], yy[:], float(W_PACK), None, OP.mult)
            nc.vector.tensor_tensor(base[:], base[:], X0w[:], op=OP.add)
            nc.vector.tensor_scalar(base[:], base[:], 0.5, None, OP.mult)
            bf = floor_of(base[:], meta, 256, nm + "fh")
            ii = meta.tile([128, 256], dt.int16, tag=nm + "w")
            nc.vector.tensor_copy(out=ii[:], in_=bf[:])
            IDXW.append(ii)
        I0W, I1W = IDXW

        if debug_taps:
            nc.sync.dma_start(taps["t_X0"].ap(), X0[:])
            nc.sync.dma_start(taps["t_QX"].ap(), QX[:])
            nc.sync.dma_start(taps["t_W3AC"].ap(),
                              W3AC[:].rearrange("p s c -> p (s c)"))
            I0f = meta.tile([128, 256], dt.float32)
            nc.vector.tensor_copy(out=I0f[:], in_=I0W[:])
            nc.sync.dma_start(taps["t_I0w"].ap(), I0f[:])
            nc.sync.dma_start(taps["t_TB"].ap(), TB[0:128, :])

        # overlapped pair-row view of TBEV: row v = elements [v*256, v*256+384)
        tb_pairs = AP(TB[:].tensor, TB[:].offset, [[256, VPAIR], [1, 384]])

        # ---- phase B: gather + blend + store ----
        for cb in range(NBCH):
            wsl = slice(cb * (BCH // 16), (cb + 1) * (BCH // 16))
            bsl = slice(cb * GPC, (cb + 1) * GPC)
            G0 = gat.tile([128, GPC, 3, 128], dt.float32, tag="G0")
            nc.gpsimd.dma_gather(
                out_ap=G0[:].rearrange("p g t c -> p g (t c)"),
                in_ap=tb_pairs, idxs_ap=I0W[:, wsl],
                num_idxs=BCH, num_idxs_reg=BCH, elem_size=384, elem_step=256)
            G1 = gat.tile([128, GPC, 3, 128], dt.float32, tag="G1")
            nc.gpsimd.dma_gather(
                out_ap=G1[:].rearrange("p g t c -> p g (t c)"),
                in_ap=tb_pairs, idxs_ap=I1W[:, wsl],
                num_idxs=BCH, num_idxs_reg=BCH, elem_size=384, elem_step=256)

            P0 = blend.tile([128, GPC, 3, 128], dt.float32, tag="P0")
            w3ac_b = W3AC[:, bsl, :, None].to_broadcast((128, GPC, 3, 128))
            nc.vector.tensor_tensor(P0[:], G0[:], w3ac_b, op=OP.mult)
            P1 = blend.tile([128, GPC, 3, 128], dt.float32, tag="P1")
            w3bd_b = W3BD[:, bsl, :, None].to_broadcast((128, GPC, 3, 128))
            nc.vector.tensor_tensor(P1[:], G1[:], w3bd_b, op=OP.mult)
            S = blend.tile([128, GPC, 3, 128], dt.float32, tag="S")
            nc.vector.tensor_tensor(S[:], P0[:], P1[:], op=OP.add)
            OUTG = blend.tile([128, GPC, 128], dt.float32, tag="OUTG")
            nc.vector.tensor_tensor(OUTG[:], S[:, :, 0, :], S[:, :, 1, :], op=OP.add)
            nc.vector.tensor_tensor(OUTG[:], OUTG[:], S[:, :, 2, :], op=OP.add)

            if debug_taps and cb == 0:
                nc.sync.dma_start(taps["t_G0"].ap(), G0[:, 0, :, :]
                                  .rearrange("p t c -> p (t c)"))

            nc.sync.dma_start(out_r[cb], OUTG[:])

    nc.compile()
    return nc


def _get(scale: float, ybias: float):
    key = (round(scale, 9), round(ybias, 9))
    if key not in _compiled:
        _compiled[key] = _build(scale, ybias)
    return _compiled[key]


def _prepare_in_maps(keypoints: np.ndarray, bev_features: np.ndarray):
    kp = np.ascontiguousarray(keypoints, dtype=np.float32)
    bev = np.asarray(bev_features, dtype=np.float32)
    in_maps = []
    for core in range(N_CORES):
        b, ch = core // 2, core % 2
        sl = slice(ch * 128, (ch + 1) * 128)
        slab = np.zeros((128, FLAT_PAD), dtype=np.float32)
        slab[:, :FLAT] = bev[b, sl, :, :W_PACK].reshape(128, FLAT)
        in_maps.append({"slab": slab, "kp": kp[b]})
    return in_maps


def _assemble(results) -> np.ndarray:
    out = np.empty((B, N, C), dtype=np.float32)
    for core in range(N_CORES):
        b, ch = core // 2, core % 2
        out[b, :, ch * 128:(ch + 1) * 128] = np.asarray(results[core]["out"])
    return out


def _scale_bias(bev_stride):
    stride = float(np.asarray(bev_stride))
    scale = 1.0 / (0.05 * stride)
    return scale, 40.0 * scale


def kernel(keypoints: np.ndarray, bev_features: np.ndarray, bev_stride) -> np.ndarray:
    from concourse.bass_utils import run_bass_kernel_spmd

    scale, ybias = _scale_bias(bev_stride)
    nc = _get(scale, ybias)
    in_maps = _prepare_in_maps(keypoints, bev_features)
    res = run_bass_kernel_spmd(nc, in_maps, list(range(N_CORES))).results
    return _assemble(res)
